# revision 2
# baseline (speedup 1.0000x reference)
"""EntangledPairsEngine Trainium2 kernel.

Sharding: data-parallel over cells/pairs across 8 NeuronCores. Core c owns
cells [128c, 128c+128) == faction c, and pairs [64c, 64c+64). Small weights
are replicated. Host does input layout prep (transposes, dtype casts, tanh/
sigmoid of raw inputs fused into the upload pass) and the tiny cross-core
finalize (global opinion, debate row update, softmax head) during unshard.

Device (per core, feature-major dense phase + row-packed bell phase):
  - MLP a/g + tension + GRU in f32 on PE/ACT/DVE.
  - Bell matvecs: both directions as fp16 moving-operand matmuls streaming
    tanh(rot) (natural + transposed HBM copies), 4 pairs packed per PSUM bank
    at partitions {0,32,64,96} via tile_position; evacuated by DVE, compacted
    to cell-major rows with SBUF->SBUF DMAs.
  - e-weighted combine, faction mean (ones-matmul) and sync in f32.
"""
import numpy as np

import concourse.bass as bass
from concourse import bacc
import concourse.mybir as mybir
from concourse.tile import TileContext
from concourse.bass_utils import run_bass_kernel_spmd

F16 = mybir.dt.float16
F32 = mybir.dt.float32
AF = mybir.ActivationFunctionType
OP = mybir.AluOpType

N_CORES = 8
N_CELLS = 1024
IN_DIM = 256
HID = 512
OUT_DIM = 256
INNER = 128
N_PAIRS = 512
N_FACTIONS = 8
SYNC = 0.15
DEBATE = 0.15

CPC = N_CELLS // N_CORES      # 128 cells per core
PPC = N_PAIRS // N_CORES      # 64 pairs per core

_CACHE = {}


def _build(repeat=1):
    nc = bacc.Bacc()

    # ---- dram I/O ----
    cmbT = nc.dram_tensor("cmbT", [IN_DIM + HID, CPC], F32, kind="ExternalInput")
    w1aT = nc.dram_tensor("w1aT", [IN_DIM + HID, INNER], F32, kind="ExternalInput")
    w1gT = nc.dram_tensor("w1gT", [IN_DIM + HID, INNER], F32, kind="ExternalInput")
    b1a = nc.dram_tensor("b1a", [INNER, 1], F32, kind="ExternalInput")
    b1g = nc.dram_tensor("b1g", [INNER, 1], F32, kind="ExternalInput")
    w2aT = nc.dram_tensor("w2aT", [INNER, OUT_DIM], F32, kind="ExternalInput")
    w2gT = nc.dram_tensor("w2gT", [INNER, OUT_DIM], F32, kind="ExternalInput")
    b2d = nc.dram_tensor("b2d", [OUT_DIM, 1], F32, kind="ExternalInput")
    wihT = nc.dram_tensor("wihT", [OUT_DIM + 1, 3 * HID], F32, kind="ExternalInput")
    whhT = nc.dram_tensor("whhT", [HID, 3 * HID], F32, kind="ExternalInput")
    brz = nc.dram_tensor("brz", [2 * HID, 1], F32, kind="ExternalInput")
    bihn = nc.dram_tensor("bihn", [HID, 1], F32, kind="ExternalInput")
    bhhn = nc.dram_tensor("bhhn", [HID, 1], F32, kind="ExternalInput")
    ebs = nc.dram_tensor("ebs", [CPC, 1], F32, kind="ExternalInput")   # sign*(1-e) per cell
    es = nc.dram_tensor("es", [CPC, 1], F32, kind="ExternalInput")     # sign*e per cell
    eye32 = nc.dram_tensor("eye32", [128, 128], F32, kind="ExternalInput")
    rotn = nc.dram_tensor("rotn", [PPC, HID, HID], F16, kind="ExternalInput")
    rott = nc.dram_tensor("rott", [PPC, HID, HID], F16, kind="ExternalInput")

    h_out = nc.dram_tensor("h_out", [CPC, HID], F32, kind="ExternalOutput")
    outT_out = nc.dram_tensor("outT_out", [OUT_DIM, CPC], F32, kind="ExternalOutput")
    tens_out = nc.dram_tensor("tens_out", [1, CPC], F32, kind="ExternalOutput")
    fmean_out = nc.dram_tensor("fmean_out", [1, HID], F32, kind="ExternalOutput")

    with TileContext(nc) as tc:
        with tc.tile_pool(name="const", bufs=1) as cp, \
             tc.tile_pool(name="work", bufs=1) as wp, \
             tc.tile_pool(name="rotp", bufs=1) as rp, \
             tc.tile_pool(name="ps", bufs=4, space="PSUM") as ps:

            # ---- load constants ----
            cmb_sb = cp.tile([128, 768], F32)
            nc.sync.dma_start(cmb_sb.rearrange("p (k m) -> p k m", k=6),
                              cmbT.rearrange("(k p) m -> p k m", p=128))
            w1a_sb = cp.tile([128, 768], F32)
            nc.sync.dma_start(w1a_sb.rearrange("p (k m) -> p k m", k=6),
                              w1aT.rearrange("(k p) m -> p k m", p=128))
            w1g_sb = cp.tile([128, 768], F32)
            nc.sync.dma_start(w1g_sb.rearrange("p (k m) -> p k m", k=6),
                              w1gT.rearrange("(k p) m -> p k m", p=128))
            w2a_sb = cp.tile([128, 256], F32)
            nc.sync.dma_start(w2a_sb, w2aT[:, :])
            w2g_sb = cp.tile([128, 256], F32)
            nc.sync.dma_start(w2g_sb, w2gT[:, :])
            b1a_sb = cp.tile([128, 1], F32)
            nc.sync.dma_start(b1a_sb, b1a[:, :])
            b1g_sb = cp.tile([128, 1], F32)
            nc.sync.dma_start(b1g_sb, b1g[:, :])
            b2d_sb = cp.tile([128, 2], F32)
            nc.sync.dma_start(b2d_sb.rearrange("p (k m) -> p k m", k=2),
                              b2d.rearrange("(k p) m -> p k m", p=128))
            wih0_sb = cp.tile([128, 1536], F32)
            nc.sync.dma_start(wih0_sb, wihT[0:128, :])
            wih1_sb = cp.tile([128, 1536], F32)
            nc.sync.dma_start(wih1_sb, wihT[128:256, :])
            wih2_sb = cp.tile([1, 1536], F32)
            nc.sync.dma_start(wih2_sb, wihT[256:257, :])
            whh_sb = cp.tile([128, 4 * 1536], F32)
            nc.sync.dma_start(whh_sb.rearrange("p (k m) -> p k m", k=4),
                              whhT.rearrange("(k p) m -> p k m", p=128))
            brz_sb = cp.tile([128, 8], F32)
            nc.sync.dma_start(brz_sb.rearrange("p (k m) -> p k m", k=8),
                              brz.rearrange("(k p) m -> p k m", p=128))
            bihn_sb = cp.tile([128, 4], F32)
            nc.sync.dma_start(bihn_sb.rearrange("p (k m) -> p k m", k=4),
                              bihn.rearrange("(k p) m -> p k m", p=128))
            bhhn_sb = cp.tile([128, 4], F32)
            nc.sync.dma_start(bhhn_sb.rearrange("p (k m) -> p k m", k=4),
                              bhhn.rearrange("(k p) m -> p k m", p=128))
            ebs_sb = cp.tile([128, 1], F32)
            nc.sync.dma_start(ebs_sb, ebs[:, :])
            es_sb = cp.tile([128, 1], F32)
            nc.sync.dma_start(es_sb, es[:, :])
            eye_sb = cp.tile([128, 128], F32)
            nc.sync.dma_start(eye_sb, eye32[:, :])
            ones_sb = cp.tile([128, 1], F32)
            nc.vector.memset(ones_sb, 1.0)

            def body():
                # ================= dense phase (f32) =================
                h1 = {}
                for t, w1_sb, b1_sb in (("a", w1a_sb, b1a_sb), ("g", w1g_sb, b1g_sb)):
                    psum_h1 = ps.tile([128, 512], F32, tag="pA", name=f"ph1{t}")
                    for k in range(6):
                        nc.tensor.matmul(
                            psum_h1[:, 0:128],
                            lhsT=w1_sb[:, 128 * k:128 * (k + 1)],
                            rhs=cmb_sb[:, 128 * k:128 * (k + 1)],
                            start=(k == 0), stop=(k == 5),
                        )
                    h1_t = wp.tile([128, 128], F32, name=f"h1{t}")
                    nc.scalar.activation(h1_t, psum_h1[:, 0:128], AF.Relu, bias=b1_sb)
                    h1[t] = h1_t

                outTb = []
                for m in range(2):
                    pa = ps.tile([128, 512], F32, tag="pA", name=f"pa{m}")
                    nc.tensor.matmul(pa[:, 0:128], lhsT=w2a_sb[:, 128 * m:128 * (m + 1)],
                                     rhs=h1["a"], start=True, stop=True)
                    pg = ps.tile([128, 512], F32, tag="pB", name=f"pg{m}")
                    nc.tensor.matmul(pg[:, 0:128], lhsT=w2g_sb[:, 128 * m:128 * (m + 1)],
                                     rhs=h1["g"], start=True, stop=True)
                    o = wp.tile([128, 128], F32, name=f"o{m}")
                    nc.vector.tensor_scalar(o, pa[:, 0:128], b2d_sb[:, m:m + 1], None, OP.add)
                    ob = wp.tile([128, 128], F32, name=f"ob{m}")
                    nc.vector.tensor_tensor(ob, o, pg[:, 0:128], OP.subtract)
                    outTb.append(ob)

                # tension = mean(output^2) over features
                pt = ps.tile([128, 512], F32, tag="pB", name="pt")
                for m in range(2):
                    sq = wp.tile([128, 128], F32, tag="sq", name=f"sq{m}")
                    nc.vector.tensor_tensor(sq, outTb[m], outTb[m], OP.mult)
                    nc.tensor.matmul(pt[0:1, 0:128], lhsT=ones_sb, rhs=sq,
                                     start=(m == 0), stop=(m == 1))
                tens = wp.tile([1, 128], F32, name="tens")
                nc.scalar.mul(tens, pt[0:1, 0:128], 1.0 / OUT_DIM)

                # ================= GRU =================
                def gmm(psum, mm, with_ih, with_hh):
                    first = True
                    if with_ih:
                        for ksb, rhs in ((wih0_sb, outTb[0]), (wih1_sb, outTb[1])):
                            nc.tensor.matmul(psum[:, 0:128],
                                             lhsT=ksb[:, 128 * mm:128 * (mm + 1)],
                                             rhs=rhs, start=first, stop=False)
                            first = False
                        nc.tensor.matmul(psum[0:128, 0:128],
                                         lhsT=wih2_sb[0:1, 128 * mm:128 * (mm + 1)],
                                         rhs=tens, start=first,
                                         stop=(not with_hh))
                        first = False
                    if with_hh:
                        for k in range(4):
                            nc.tensor.matmul(psum[:, 0:128],
                                             lhsT=whh_sb[:, 1536 * k + 128 * mm:1536 * k + 128 * (mm + 1)],
                                             rhs=cmb_sb[:, 256 + 128 * k:256 + 128 * (k + 1)],
                                             start=first, stop=(k == 3))
                            first = False

                gates = {}
                for mm in range(8):      # r: 0-3, z: 4-7
                    pgm = ps.tile([128, 512], F32, tag="pA", name=f"pgm{mm}")
                    gmm(pgm, mm, True, True)
                    gt = wp.tile([128, 128], F32, tag="gate", bufs=8, name=f"gate{mm}")
                    nc.scalar.activation(gt, pgm[:, 0:128], AF.Sigmoid,
                                         bias=brz_sb[:, mm:mm + 1])
                    gates[mm] = gt

                nh32 = []
                nh16 = []
                for j in range(4):       # n slices: mm = 8 + j
                    mm = 8 + j
                    pin = ps.tile([128, 512], F32, tag="pA", name=f"pin{j}")
                    gmm(pin, mm, True, False)
                    phn = ps.tile([128, 512], F32, tag="pB", name=f"phn{j}")
                    gmm(phn, mm, False, True)
                    hn = wp.tile([128, 128], F32, tag="hn", name=f"hn{j}")
                    nc.vector.tensor_scalar(hn, phn[:, 0:128], bhhn_sb[:, j:j + 1], None, OP.add)
                    inj = wp.tile([128, 128], F32, tag="inj", name=f"inj{j}")
                    nc.vector.tensor_scalar(inj, pin[:, 0:128], bihn_sb[:, j:j + 1], None, OP.add)
                    rh = wp.tile([128, 128], F32, tag="rh", name=f"rh{j}")
                    nc.vector.tensor_tensor(rh, gates[j], hn, OP.mult)
                    arg = wp.tile([128, 128], F32, tag="arg", name=f"arg{j}")
                    nc.vector.tensor_tensor(arg, inj, rh, OP.add)
                    nj = wp.tile([128, 128], F32, tag="nj", name=f"nj{j}")
                    nc.scalar.activation(nj, arg, AF.Tanh)
                    # new_h = n + z*(h - n)
                    tj = wp.tile([128, 128], F32, tag="tj", name=f"tj{j}")
                    nc.vector.tensor_tensor(tj, cmb_sb[:, 256 + 128 * j:256 + 128 * (j + 1)], nj, OP.subtract)
                    t2 = wp.tile([128, 128], F32, tag="t2", name=f"t2{j}")
                    nc.vector.tensor_tensor(t2, gates[4 + j], tj, OP.mult)
                    nh = wp.tile([128, 128], F32, bufs=1, name=f"nh{j}")
                    nc.vector.tensor_tensor(nh, nj, t2, OP.add)
                    nh32.append(nh)
                    n16 = wp.tile([128, 128], F16, bufs=1, name=f"n16_{j}")
                    nc.vector.tensor_copy(n16, nh)
                    nh16.append(n16)

                # cell-major h (f32): hcm[cell, feat]
                hcm = wp.tile([128, 512], F32, bufs=1, name="hcm")
                for j in range(4):
                    ptr = ps.tile([128, 512], F32, tag="pB", name=f"ptr{j}")
                    nc.tensor.transpose(ptr[:, 0:128], nh32[j], eye_sb)
                    nc.vector.tensor_copy(hcm[:, 128 * j:128 * (j + 1)], ptr[:, 0:128])

                # ================= bell phase =================
                yall = wp.tile([128, 512], F32, bufs=1, name="yall")
                for g in range(16):
                    pyi = ps.tile([128, 512], F32, tag="pA", name=f"pyi{g}")
                    pyj = ps.tile([128, 512], F32, tag="pB", name=f"pyj{g}")
                    for sp in range(4):
                        p = 4 * g + sp
                        rn = rp.tile([128, 2048], F16, tag="rn", bufs=9, name=f"rn{p}")
                        nc.sync.dma_start(
                            rn.rearrange("q (al b) -> q al b", al=4),
                            rotn[p].rearrange("(al q) b -> q al b", q=128))
                        rt = rp.tile([128, 2048], F16, tag="rt", bufs=9, name=f"rt{p}")
                        nc.sync.dma_start(
                            rt.rearrange("q (al b) -> q al b", al=4),
                            rott[p].rearrange("(al q) b -> q al b", q=128))
                        ci, cj = 2 * (p % 64), 2 * (p % 64) + 1
                        for al in range(4):
                            # y_i[a] = sum_b T[a,b] h_j[b]; rott chunk al = T^T rows [b-chunk al]
                            nc.tensor.matmul(
                                pyi[32 * sp:32 * sp + 1, :],
                                lhsT=nh16[al][:, cj:cj + 1],
                                rhs=rt[:, 512 * al:512 * (al + 1)],
                                start=(al == 0), stop=(al == 3),
                                tile_position=(0, 32 * sp))
                        for al in range(4):
                            # y_j[b] = sum_a T[a,b] h_i[a]
                            nc.tensor.matmul(
                                pyj[32 * sp:32 * sp + 1, :],
                                lhsT=nh16[al][:, ci:ci + 1],
                                rhs=rn[:, 512 * al:512 * (al + 1)],
                                start=(al == 0), stop=(al == 3),
                                tile_position=(0, 32 * sp))
                    evi = wp.tile([128, 512], F32, tag="evi", bufs=3, name=f"evi{g}")
                    nc.vector.tensor_copy(evi, pyi)
                    evj = wp.tile([128, 512], F32, tag="evj", bufs=3, name=f"evj{g}")
                    nc.vector.tensor_copy(evj, pyj)
                    # compact rows {0,32,64,96} -> yall rows {8g(+1), +2, +4, +6}
                    si = bass.AP(evi.tensor, evi.offset, [[32 * 512, 4], [1, 512]])
                    di = bass.AP(yall.tensor, yall.offset + (8 * g) * 512, [[2 * 512, 4], [1, 512]])
                    nc.sync.dma_start(di, si)
                    sj = bass.AP(evj.tensor, evj.offset, [[32 * 512, 4], [1, 512]])
                    dj = bass.AP(yall.tensor, yall.offset + (8 * g + 1) * 512, [[2 * 512, 4], [1, 512]])
                    nc.sync.dma_start(dj, sj)

                # ================= combine + faction sync =================
                c1 = wp.tile([128, 512], F32, bufs=1, name="c1")
                nc.vector.tensor_scalar(c1, hcm, ebs_sb, None, OP.mult)
                c2 = wp.tile([128, 512], F32, bufs=1, name="c2")
                nc.vector.tensor_scalar(c2, yall, es_sb, None, OP.mult)
                hmix = wp.tile([128, 512], F32, bufs=1, name="hmix")
                nc.vector.tensor_tensor(hmix, c1, c2, OP.add)

                pfm = ps.tile([128, 512], F32, tag="pA", name="pfm")
                nc.tensor.matmul(pfm[0:1, :], lhsT=ones_sb, rhs=hmix, start=True, stop=True)
                fm = wp.tile([1, 512], F32, bufs=1, name="fm")
                nc.vector.tensor_copy(fm, pfm[0:1, :])
                fms = wp.tile([1, 512], F32, bufs=1, name="fms")
                nc.vector.tensor_scalar(fms, fm, SYNC / CPC, None, OP.mult)
                fmb = wp.tile([128, 512], F32, bufs=1, name="fmb")
                nc.sync.dma_start(
                    fmb, bass.AP(fms.tensor, fms.offset, [[512, 1], [0, 128], [1, 512]]))
                hr = wp.tile([128, 512], F32, bufs=1, name="hr")
                nc.vector.tensor_scalar(hr, hmix, 1.0 - SYNC, None, OP.mult)
                hrf = wp.tile([128, 512], F32, bufs=1, name="hrf")
                nc.vector.tensor_tensor(hrf, hr, fmb, OP.add)
                fmo = wp.tile([1, 512], F32, bufs=1, name="fmo")
                nc.vector.tensor_scalar(fmo, fm, 1.0 / CPC, None, OP.mult)

                nc.gpsimd.dma_start(h_out[:, :], hrf)
                nc.gpsimd.dma_start(outT_out[0:128, :], outTb[0])
                nc.gpsimd.dma_start(outT_out[128:256, :], outTb[1])
                nc.gpsimd.dma_start(tens_out[:, :], tens)
                nc.gpsimd.dma_start(fmean_out[:, :], fmo)

            if repeat == 1:
                body()
            else:
                with tc.For_i(0, repeat, 1):
                    body()
    nc.finalize()
    return nc


def _sigmoid(x):
    return 1.0 / (1.0 + np.exp(-x))


def _prep(inputs):
    """Build per-core in_maps from full inputs."""
    x = np.asarray(inputs["x"], np.float32)
    hiddens = np.asarray(inputs["hiddens"], np.float32)
    W1a = np.asarray(inputs["W1a"], np.float32)
    b1a = np.asarray(inputs["b1a"], np.float32)
    W2a = np.asarray(inputs["W2a"], np.float32)
    b2a = np.asarray(inputs["b2a"], np.float32)
    W1g = np.asarray(inputs["W1g"], np.float32)
    b1g = np.asarray(inputs["b1g"], np.float32)
    W2g = np.asarray(inputs["W2g"], np.float32)
    b2g = np.asarray(inputs["b2g"], np.float32)
    w_ih = np.asarray(inputs["w_ih"], np.float32)
    w_hh = np.asarray(inputs["w_hh"], np.float32)
    b_ih = np.asarray(inputs["b_ih"], np.float32)
    b_hh = np.asarray(inputs["b_hh"], np.float32)
    ent = np.asarray(inputs["entanglement"], np.float32)
    rot = np.asarray(inputs["bell_rotations"], np.float32)

    shared = {
        "w1aT": np.ascontiguousarray(W1a.T),
        "w1gT": np.ascontiguousarray(W1g.T),
        "b1a": b1a.reshape(INNER, 1).copy(),
        "b1g": b1g.reshape(INNER, 1).copy(),
        "w2aT": np.ascontiguousarray(W2a.T),
        "w2gT": np.ascontiguousarray(W2g.T),
        "b2d": (b2a - b2g).reshape(OUT_DIM, 1).copy(),
        "wihT": np.ascontiguousarray(w_ih.T),
        "whhT": np.ascontiguousarray(w_hh.T),
        "brz": (b_ih + b_hh)[:2 * HID].reshape(-1, 1).copy(),
        "bihn": b_ih[2 * HID:].reshape(-1, 1).copy(),
        "bhhn": b_hh[2 * HID:].reshape(-1, 1).copy(),
        "eye32": np.eye(128, dtype=np.float32),
    }

    e = _sigmoid(ent.astype(np.float64)).astype(np.float32)     # [512]
    sign = np.tile(np.array([1.0, -1.0], np.float32), CPC // 2)

    xb = np.broadcast_to(x.reshape(IN_DIM, 1), (IN_DIM, CPC))
    tanh_rot = np.tanh(rot)     # [512, 512, 512] f32

    in_maps = []
    for c in range(N_CORES):
        cells = slice(CPC * c, CPC * (c + 1))
        pairs = slice(PPC * c, PPC * (c + 1))
        e_cell = np.repeat(e[pairs], 2)     # [128]
        m = dict(shared)
        m["cmbT"] = np.ascontiguousarray(
            np.concatenate([xb, hiddens[cells].T], axis=0))
        m["ebs"] = (sign * (1.0 - e_cell)).reshape(CPC, 1).copy()
        m["es"] = (sign * e_cell).reshape(CPC, 1).copy()
        tr = tanh_rot[pairs]
        m["rotn"] = np.ascontiguousarray(tr, dtype=np.float16)
        m["rott"] = np.ascontiguousarray(tr.transpose(0, 2, 1), dtype=np.float16)
        in_maps.append(m)
    return in_maps


def _run(inputs, repeat=1):
    key = repeat
    if key not in _CACHE:
        _CACHE[key] = _build(repeat)
    nc = _CACHE[key]
    in_maps = _prep(inputs)
    res = run_bass_kernel_spmd(nc, in_maps, core_ids=list(range(N_CORES)))
    return res.results


def kernel(**inputs):
    step = int(np.asarray(inputs["step"]))
    head_w = np.asarray(inputs["head_w"], np.float32)
    head_b = np.asarray(inputs["head_b"], np.float32)

    results = _run(inputs, repeat=1)

    h_full = np.concatenate([r["h_out"] for r in results], axis=0)          # [1024, 512]
    fmeans = np.concatenate([r["fmean_out"] for r in results], axis=0)      # [8, 512]
    out_full = np.concatenate([r["outT_out"].T for r in results], axis=0)   # [1024, 256]
    tens_full = np.concatenate([r["tens_out"][0] for r in results], axis=0)  # [1024]

    if step > 5:
        go = fmeans.mean(axis=0)
        dc = max(1, CPC // 4)
        for c in range(N_CORES):
            r0 = CPC * c
            h_full[r0:r0 + dc] = (1.0 - DEBATE) * h_full[r0:r0 + dc] + DEBATE * go

    t = tens_full - tens_full.max()
    w = np.exp(t)
    w = w / w.sum()
    combined = w @ out_full                                  # [256]
    pred = (combined @ head_w.T + head_b).reshape(1, IN_DIM).astype(np.float32)
    avg_tension = np.float32(tens_full.mean())
    return pred, avg_tension, h_full.astype(np.float32)
